# revision 21
# baseline (speedup 1.0000x reference)
"""Trainium2 Bass kernel for single-query multi-head attention.

Reference computation (B=32, N=4096, D=1024, H=16, dk=dv=64):
    q = (query @ wq).reshape(B, H, dk)          # [B, H, dk]
    k = (key @ wk).reshape(B, N, H, dk)
    v = (value @ wv).reshape(B, N, H, dv)
    scores = einsum("bhd,bnhd->bhn", q, k) / 8
    attn = softmax(scores, axis=-1)
    out = einsum("bhn,bnhd->bhd", attn, v).reshape(B, H*dv)

Algebraic restructuring (64x FLOP reduction vs naive):
    scores[b,n,h] = key[b,n,:] . R_b[:,h]   where R_b[:,h] = wk[:,h-blk] @ q4[b,h-blk]
    out[b,h-blk]  = (attn[b,h,:] @ value[b]) @ wv[:,h-blk]
so the huge key/value projections ([B,N,D]@[D,D]) are never materialized.

Layout/precision strategy (vs the first working version):
  * key is shipped to DRAM PRE-TRANSPOSED per batch ([D, N] tiles) in
    fp8-e4m3: halves the dominant key HBM stream and removes every
    on-chip key transpose.  scores^T[h, n] is computed directly with the
    tiny R as the stationary operand and k^T streaming.
  * R is kept in split-fp8 (hi + residual lo), so the scores matmul can
    run in fp8 DoubleRow perf mode (2 contraction-subtiles per pass,
    0.5 cyc/row).  Verified numerically: rel-err 1.16e-2 (vs 1.14e-2
    with a bf16 R); value in fp8 would blow the 2e-2 budget, so the
    attn@v pass stays bf16.
  * wk is shipped pre-transposed (wkT) so the R prologue needs no
    on-chip wk transposes either.

Sharding: data-parallel over batch, 4 batch elements per core, 8 cores,
no collectives. Each core streams 16.8 MB fp8 key + 33.5 MB bf16 value.
"""

import os
import sys

for _p in ("/opt/trn_rl_repo", os.path.expanduser("~/.axon_site/_ro/trn_rl_repo")):
    if os.path.isdir(_p) and _p not in sys.path:
        sys.path.insert(0, _p)

import numpy as np
from contextlib import ExitStack

from concourse import bass, bacc, mybir, tile, masks
from concourse.bass_utils import run_bass_kernel_spmd

N_CORES = 8
B, N, D = 32, 4096, 1024
H, DK = 16, 64
BL = B // N_CORES          # 4 batch elements per core
NTK = 2048                 # key rows per kT DMA tile (2KB runs per d-line)
NTV = 512                  # value rows per DMA tile
NG = 512                   # rows per compute group (one scores PSUM tile)
F32 = mybir.dt.float32
BF16 = mybir.dt.bfloat16
FP8 = mybir.dt.float8e4
EXP = mybir.ActivationFunctionType.Exp
DR = mybir.MatmulPerfMode.DoubleRow


def build_graph(debug=False):
    nc = bacc.Bacc()
    q_ext = nc.declare_dram_parameter("query", [BL, D], F32, isOutput=False)
    # pre-transposed fp8 key: kx[b, t, d, r] = key[b, t*NTK + r, d]
    kx_ext = nc.declare_dram_parameter("kx", [BL, N // NTK, D, NTK], FP8,
                                       isOutput=False)
    v_ext = nc.declare_dram_parameter("value", [BL, N, D], BF16, isOutput=False)
    # weights pre-arranged host-side into the SBUF layout
    # w_dev[p, jc*D + k] = w[jc*128 + p, k] so the load is a dense 2D copy
    # (the partition-gather layout costs ~10us of descriptor generation)
    wq_ext = nc.declare_dram_parameter("wq", [128, 8 * D], BF16, isOutput=False)
    # pre-transposed wk: wkT[hk, d] = wk[d, hk], same p-major device layout
    wkt_ext = nc.declare_dram_parameter("wkT", [128, 8 * D], BF16, isOutput=False)
    wv_ext = nc.declare_dram_parameter("wv", [128, 8 * D], BF16, isOutput=False)
    out_ext = nc.declare_dram_parameter("out", [BL, D], F32, isOutput=True)
    dbg = None
    if debug:
        dbg = {
            "q4": nc.declare_dram_parameter("dbg_q4", [BL, D], F32, isOutput=True),
            "r4t": nc.declare_dram_parameter("dbg_r4t", [BL * H, D], F32, isOutput=True),
            "sct": nc.declare_dram_parameter("dbg_sct", [H, NG], F32, isOutput=True),
            "et": nc.declare_dram_parameter("dbg_et", [H, NG], F32, isOutput=True),
            "shat": nc.declare_dram_parameter("dbg_shat", [H, D], F32, isOutput=True),
        }

    with ExitStack() as ctx:
        tc = ctx.enter_context(tile.TileContext(nc))
        _body(ctx, tc, nc, q_ext, kx_ext, v_ext, wq_ext, wkt_ext, wv_ext, out_ext,
              dbg=dbg)
    return nc


def _body(ctx, tc, nc, q_ext, kx_ext, v_ext, wq_ext, wkt_ext, wv_ext, out_ext,
          dbg=None):
    const_pool = ctx.enter_context(tc.tile_pool(name="const", bufs=1))
    r_pool = ctx.enter_context(tc.tile_pool(name="rpool", bufs=1))
    st_pool = ctx.enter_context(tc.tile_pool(name="st", bufs=1))
    wstream = ctx.enter_context(tc.tile_pool(name="wstream", bufs=2))
    kx_pool = ctx.enter_context(tc.tile_pool(name="kxld", bufs=3))
    val_pool = ctx.enter_context(tc.tile_pool(name="valld", bufs=5))
    et_pool = ctx.enter_context(tc.tile_pool(name="etp", bufs=2))
    e_pool = ctx.enter_context(tc.tile_pool(name="ep", bufs=2))
    small_pool = ctx.enter_context(tc.tile_pool(name="small", bufs=1))
    ps_a = ctx.enter_context(tc.tile_pool(name="ps_a", bufs=2, space="PSUM"))
    ps_sc = ctx.enter_context(tc.tile_pool(name="ps_sc", bufs=3, space="PSUM"))
    ps_acc = ctx.enter_context(tc.tile_pool(name="ps_acc", bufs=1, space="PSUM"))

    ident_f = const_pool.tile([128, 128], F32, tag="idf")
    masks.make_identity(nc, ident_f[:])
    ident_b = const_pool.tile([128, 128], BF16, tag="idb")
    masks.make_identity(nc, ident_b[:])

    # ---------------- prologue: q-projection ----------------
    # Single-shot dense weight loads, one per queue so they land in parallel
    # ahead of the key/value streams (per-queue FIFO puts them first).
    wq_sb = wstream.tile([128, 8 * D], BF16, tag="wq", bufs=1)
    nc.sync.dma_start(wq_sb[:], wq_ext[:])
    wkt_sb = wstream.tile([128, 8 * D], BF16, tag="wkt", bufs=1)
    nc.gpsimd.dma_start(wkt_sb[:], wkt_ext[:])

    # query [BL, D] -> qT chunks [128, BL] (contraction dim on partitions)
    q_sb = small_pool.tile([BL, D], F32, tag="q")
    nc.sync.dma_start(q_sb[:], q_ext[:])
    qT = small_pool.tile([128, 8 * BL], BF16, tag="qT")
    for jc in range(8):
        pt = ps_a.tile([128, 128], F32, tag="a")
        nc.tensor.transpose(pt[:, :BL], q_sb[:, jc * 128:(jc + 1) * 128],
                            ident_f[:BL, :BL])
        nc.any.tensor_copy(qT[:, jc * BL:(jc + 1) * BL], pt[:, :BL])

    # q4[b, hk] = sum_j query[b, j] * wq[j, hk]   (all 4 batches at once)
    q4_ps = ps_acc.tile([BL, D], F32, tag="acc")
    for jc in range(8):
        for half in range(2):
            nc.tensor.matmul(q4_ps[:, half * 512:(half + 1) * 512],
                             qT[:, jc * BL:(jc + 1) * BL],
                             wq_sb[:, jc * D + half * 512:jc * D + (half + 1) * 512],
                             start=(jc == 0), stop=(jc == 7))
    q4_sb = small_pool.tile([BL, D], F32, tag="q4")
    nc.any.tensor_copy(q4_sb[:], q4_ps[:])
    if dbg:
        nc.sync.dma_start(dbg["q4"][:], q4_sb[:])

    # q4T chunks: [128 hk, BL]
    q4T = small_pool.tile([128, 8 * BL], BF16, tag="q4T")
    for hc in range(8):
        pt = ps_a.tile([128, 128], F32, tag="a")
        nc.tensor.transpose(pt[:, :BL], q4_sb[:, hc * 128:(hc + 1) * 128],
                            ident_f[:BL, :BL])
        nc.any.tensor_copy(q4T[:, hc * BL:(hc + 1) * BL], pt[:, :BL])

    # Block-diagonal q: Qbd[hk, b*H + h] = q4[b, hk] iff h == hk // 64
    qbd = []
    for hc in range(8):
        qb = small_pool.tile([128, BL * H], BF16, tag=f"qbd{hc}", name=f"qbd{hc}")
        nc.vector.memset(qb[:], 0.0)
        nc.vector.tensor_copy(qb[0:64, 2 * hc:BL * H:H],
                              q4T[0:64, hc * BL:(hc + 1) * BL])
        nc.vector.tensor_copy(qb[64:128, 2 * hc + 1:BL * H:H],
                              q4T[64:128, hc * BL:(hc + 1) * BL])
        qbd.append(qb)

    # R4T[b*H + h, d] = sum_hk Qbd[hk, b*H+h] * wkT[hk, d]
    r4T_ps = ps_acc.tile([BL * H, D], F32, tag="acc")
    for hc in range(8):
        for half in range(2):
            nc.tensor.matmul(r4T_ps[:, half * 512:(half + 1) * 512],
                             qbd[hc][:],
                             wkt_sb[:, hc * D + half * 512:hc * D + (half + 1) * 512],
                             start=(hc == 0), stop=(hc == 7))
    r4T_sb = small_pool.tile([BL * H, D], F32, tag="r4T")
    nc.any.tensor_copy(r4T_sb[:], r4T_ps[:])
    if dbg:
        nc.sync.dma_start(dbg["r4t"][:], r4T_sb[:])

    # Split-fp8 R in DoubleRow layout: per d-chunk, [128 d, 2, BL*H] with
    # ksub 0 = Rhi = fp8(R) and ksub 1 = Rlo = fp8(R - Rhi).  One DoubleRow
    # matmul per chunk then computes (Rhi + Rlo)^T @ kx by feeding the same
    # kx chunk to both ksubs (stride-0 broadcast).
    rcat = [r_pool.tile([128, 2, BL * H], FP8, tag=f"rc{dc}", name=f"rc{dc}")
            for dc in range(8)]
    for dc in range(8):
        pt = ps_a.tile([128, 128], F32, tag="a")
        nc.tensor.transpose(pt[:, :BL * H], r4T_sb[:, dc * 128:(dc + 1) * 128],
                            ident_f[:BL * H, :BL * H])
        nc.vector.tensor_copy(rcat[dc][:, 0, :], pt[:, :BL * H])
        # residual: R - fp8(R), computed in f32 then cast to fp8
        rhi_f = small_pool.tile([128, BL * H], F32, tag="rhif")
        nc.vector.tensor_copy(rhi_f[:], rcat[dc][:, 0, :])
        rlo_f = small_pool.tile([128, BL * H], F32, tag="rlof")
        nc.vector.tensor_sub(rlo_f[:], pt[:, :BL * H], rhi_f[:])
        nc.vector.tensor_copy(rcat[dc][:, 1, :], rlo_f[:])

    # ---------------- main loop ----------------
    sT = [st_pool.tile([128, BL * H], BF16, tag=f"st{dc}", name=f"st{dc}")
          for dc in range(8)]
    n_groups = N // NG                   # 8 groups of 512 rows per batch
    groups_per_kx = NTK // NG            # 4
    wv_sb = None
    for b in range(BL):
        vb = v_ext[b].rearrange("(t four p) d -> t p four d", four=4, p=128)
        # Two consecutive batches share the accumulator banks at partition
        # offsets 0 and 32, so batch b+1 can start accumulating while batch
        # b's epilogue still reads its slice.
        if b % 2 == 0:
            s_pair = ps_acc.tile([32 + H, D], F32, tag="acc", name=f"sacc{b}")
        po = 32 * (b % 2)
        s_ps = s_pair[po:po + H]
        esum = small_pool.tile([H, 1], F32, tag="esum", bufs=2, name=f"esum{b}")
        if b == 1:
            # wv rides the sync queue mid-stream (reusing wq's SBUF, which is
            # dead after the prologue), clear of the value-queue tail
            wv_sb = wstream.tile([128, 8 * D], BF16, tag="wq", bufs=1,
                                 name="wv")
            nc.sync.dma_start(wv_sb[:], wv_ext[:])
        for g in range(n_groups):
            if g % groups_per_kx == 0:
                kx_t = kx_pool.tile([128, 8, NTK], FP8, tag="kx")
                nc.sync.dma_start(
                    kx_t[:],
                    kx_ext[b, g // groups_per_kx].rearrange(
                        "(dc p) r -> p dc r", p=128))
            go = (g % groups_per_kx) * NG    # row offset inside kx tile
            v_t = val_pool.tile([128, 4 * D], BF16, tag="v")
            nc.gpsimd.dma_start(v_t[:].rearrange("p (four d) -> p four d", four=4),
                                vb[g])
            # scores^T [16, 512] in one PSUM bank, one DoubleRow matmul per
            # d-chunk (kx chunk broadcast to both R ksubs)
            scT = ps_sc.tile([H, NG], F32, tag="sc")
            for dc in range(8):
                nc.tensor.matmul(
                    scT[:],
                    rcat[dc][:, :, b * H:(b + 1) * H],
                    kx_t[:, dc, go:go + NG].unsqueeze(1).broadcast_to(
                        (128, 2, NG)),
                    start=(dc == 0), stop=(dc == 7), perf_mode=DR,
                    skip_group_check=True)
            # exp(scores / 8); no max-subtraction needed (|scores| < ~3)
            eT = et_pool.tile([H, NG], BF16, tag="et")
            nc.scalar.activation(eT[:], scT[:], EXP, scale=0.125)
            # running sum of exp on DVE (frees the PE sum-matmuls)
            gsum = small_pool.tile([H, 1], F32, tag="gsum", bufs=2)
            nc.vector.reduce_sum(gsum[:], eT[:], axis=mybir.AxisListType.X)
            if g == 0:
                nc.vector.tensor_copy(esum[:], gsum[:])
            else:
                nc.vector.tensor_add(esum[:], esum[:], gsum[:])
            if dbg and b == 0 and g == 0:
                sct_f = small_pool.tile([H, NG], F32, tag="sctf")
                nc.vector.tensor_copy(sct_f[:], scT[:])
                nc.sync.dma_start(dbg["sct"][:], sct_f[:])
                et_f = small_pool.tile([H, NG], F32, tag="etf")
                nc.vector.tensor_copy(et_f[:], eT[:])
                nc.sync.dma_start(dbg["et"][:], et_f[:])
            # transpose e^T -> e [128, 4*16] (sub, h)
            e_ps = ps_a.tile([128, 4 * H], BF16, tag="a")
            for s in range(4):
                nc.tensor.transpose(e_ps[:, s * H:(s + 1) * H],
                                    eT[:, s * 128:(s + 1) * 128],
                                    ident_b[:H, :H])
            e_sb = e_pool.tile([128, 4 * H], BF16, tag="e")
            nc.vector.tensor_copy(e_sb[:], e_ps[:])
            # attn @ value for the 4 row-subtiles
            for s in range(4):
                first = g == 0 and s == 0
                last = g == n_groups - 1 and s == 3
                e_s = e_sb[:, s * H:(s + 1) * H]
                nc.tensor.matmul(s_ps[:, 0:512], e_s,
                                 v_t[:, s * D:s * D + 512],
                                 start=first, stop=last, skip_group_check=True)
                nc.tensor.matmul(s_ps[:, 512:1024], e_s,
                                 v_t[:, s * D + 512:(s + 1) * D],
                                 start=first, stop=last, skip_group_check=True)
        # batch epilogue: normalize and transpose s
        recip = small_pool.tile([H, 1], F32, tag="recip")
        nc.vector.reciprocal(recip[:], esum[:])
        shat = small_pool.tile([H, D], F32, tag="shat")
        nc.vector.tensor_scalar_mul(shat[:], s_ps[:], recip[:])
        if dbg and b == 0:
            nc.sync.dma_start(dbg["shat"][:], shat[:])
        for dc in range(8):
            pt = ps_a.tile([128, 128], F32, tag="st", name="pt_st", bufs=1)
            nc.tensor.transpose(pt[:, :H], shat[:, dc * 128:(dc + 1) * 128],
                                ident_f[:H, :H])
            nc.any.tensor_copy(sT[dc][:, b * H:(b + 1) * H], pt[:, :H])

        # ---------------- output projection (per batch pair) ----------------
        # out[b, h*64+j] = sum_d sT[d, b*H+h] * wv[d, h*64+j]
        # Batches 0-1 project mid-kernel (hidden under the main stream); only
        # batches 2-3 land in the tail.  One PSUM tile (= one bank) per head:
        # a start=True matmul clears the has_written bits of its WHOLE bank,
        # so interleaved accumulation groups must never share a bank.
        if b % 2 == 1:
            o_sb = small_pool.tile([2, D], F32, tag=f"o{b}", name=f"o{b}")
            for h in range(H):
                oh_ps = ps_sc.tile([2, 64], F32, tag="sc", name=f"oh{b}_{h}")
                for dc in range(8):
                    nc.tensor.matmul(
                        oh_ps[:],
                        sT[dc][:, (b - 1) * H + h:(b + 1) * H:H],
                        wv_sb[:, dc * D + h * 64:dc * D + (h + 1) * 64],
                        start=(dc == 0), stop=(dc == 7))
                nc.any.tensor_copy(o_sb[:, h * 64:(h + 1) * 64], oh_ps[:])
            nc.sync.dma_start(out_ext[b - 1:b + 1, :], o_sb[:])


_graph_cache = {}


def _get_graph():
    if "nc" not in _graph_cache:
        nc = build_graph()
        # Bacc.finalize runs the sync-wait-splitting passes the TRN2 ISA
        # requires (<=1 wait per instruction); the pjrt path serializes the
        # module as-is, so finalize must happen before run.
        if not nc.is_finalized():
            nc.finalize()
        _graph_cache["nc"] = nc
    return _graph_cache["nc"]


def make_in_maps(query, key, value, wq, wk, wv):
    import ml_dtypes
    f = np.float32
    bf = ml_dtypes.bfloat16
    f8 = ml_dtypes.float8_e4m3
    # dtype conversion and layout (transpose/tile) happen host-side as part
    # of sharding; all arithmetic runs on device.
    key8 = np.asarray(key, dtype=np.float32).astype(f8)
    # kx[b, t, d, r] = key[b, t*NTK + r, d]
    kx = np.ascontiguousarray(
        key8.reshape(B, N // NTK, NTK, D).transpose(0, 1, 3, 2))
    value = np.ascontiguousarray(value).astype(bf)

    def pmajor(w):
        # device layout [128, 8*D]: w_dev[p, jc*D + k] = w[jc*128 + p, k]
        return np.ascontiguousarray(
            np.asarray(w, dtype=np.float32).reshape(8, 128, D)
            .transpose(1, 0, 2).reshape(128, 8 * D)).astype(bf)

    wq_b = pmajor(wq)
    wkT = pmajor(np.asarray(wk).T)
    wv_b = pmajor(wv)
    maps = []
    for c in range(N_CORES):
        sl = slice(c * BL, (c + 1) * BL)
        maps.append({
            "query": np.ascontiguousarray(query[sl], dtype=f),
            "kx": np.ascontiguousarray(kx[sl]),
            "value": np.ascontiguousarray(value[sl]),
            "wq": wq_b, "wkT": wkT, "wv": wv_b,
        })
    return maps


def kernel(query, key, value, wq, wk, wv):
    nc = _get_graph()
    in_maps = make_in_maps(query, key, value, wq, wk, wv)
    res = run_bass_kernel_spmd(nc, in_maps, core_ids=list(range(N_CORES)))
    out = np.concatenate([r["out"] for r in res.results], axis=0)
    return out.astype(np.float32)


# revision 24
# speedup vs baseline: 1.1254x; 1.1254x over previous
"""Trainium2 Bass kernel for single-query multi-head attention.

Reference computation (B=32, N=4096, D=1024, H=16, dk=dv=64):
    q = (query @ wq).reshape(B, H, dk)          # [B, H, dk]
    k = (key @ wk).reshape(B, N, H, dk)
    v = (value @ wv).reshape(B, N, H, dv)
    scores = einsum("bhd,bnhd->bhn", q, k) / 8
    attn = softmax(scores, axis=-1)
    out = einsum("bhn,bnhd->bhd", attn, v).reshape(B, H*dv)

Algebraic restructuring (64x FLOP reduction vs naive):
    scores[b,n,h] = key[b,n,:] . R_b[:,h]   where R_b[:,h] = wk[:,h-blk] @ q4[b,h-blk]
    out[b,h-blk]  = (attn[b,h,:] @ value[b]) @ wv[:,h-blk]
so the huge key/value projections ([B,N,D]@[D,D]) are never materialized.

Layout/precision strategy (vs the first working version):
  * key is shipped to DRAM PRE-TRANSPOSED per batch ([D, N] tiles) in
    fp8-e4m3: halves the dominant key HBM stream and removes every
    on-chip key transpose.  scores^T[h, n] is computed directly with the
    tiny R as the stationary operand and k^T streaming.
  * R is kept in split-fp8 (hi + residual lo), so the scores matmul can
    run in fp8 DoubleRow perf mode (2 contraction-subtiles per pass,
    0.5 cyc/row).  Verified numerically: rel-err 1.16e-2 (vs 1.14e-2
    with a bf16 R); value in fp8 would blow the 2e-2 budget, so the
    attn@v pass stays bf16.
  * wk is shipped pre-transposed (wkT) so the R prologue needs no
    on-chip wk transposes either.

Sharding: data-parallel over batch, 4 batch elements per core, 8 cores,
no collectives. Each core streams 16.8 MB fp8 key + 33.5 MB bf16 value.
"""

import os
import sys

for _p in ("/opt/trn_rl_repo", os.path.expanduser("~/.axon_site/_ro/trn_rl_repo")):
    if os.path.isdir(_p) and _p not in sys.path:
        sys.path.insert(0, _p)

import numpy as np
from contextlib import ExitStack

from concourse import bass, bacc, mybir, tile, masks
from concourse.bass_utils import run_bass_kernel_spmd

N_CORES = 8
B, N, D = 32, 4096, 1024
H, DK = 16, 64
BL = B // N_CORES          # 4 batch elements per core
NTK = 2048                 # key rows per kT DMA tile (2KB runs per d-line)
NTV = 512                  # value rows per DMA tile
NG = 512                   # rows per compute group (one scores PSUM tile)
F32 = mybir.dt.float32
BF16 = mybir.dt.bfloat16
FP8 = mybir.dt.float8e4
EXP = mybir.ActivationFunctionType.Exp
DR = mybir.MatmulPerfMode.DoubleRow


def build_graph(debug=False):
    nc = bacc.Bacc()
    q_ext = nc.declare_dram_parameter("query", [BL, D], F32, isOutput=False)
    # pre-transposed fp8 key: kx[b, t, d, r] = key[b, t*NTK + r, d]
    kx_ext = nc.declare_dram_parameter("kx", [BL, N // NTK, D, NTK], FP8,
                                       isOutput=False)
    v_ext = nc.declare_dram_parameter("value", [BL, N, D], BF16, isOutput=False)
    # weights pre-arranged host-side into the SBUF layout
    # w_dev[p, jc*D + k] = w[jc*128 + p, k] so the load is a dense 2D copy
    # (the partition-gather layout costs ~10us of descriptor generation)
    wq_ext = nc.declare_dram_parameter("wq", [128, 8 * D], BF16, isOutput=False)
    # pre-transposed wk: wkT[hk, d] = wk[d, hk], same p-major device layout
    wkt_ext = nc.declare_dram_parameter("wkT", [128, 8 * D], BF16, isOutput=False)
    wv_ext = nc.declare_dram_parameter("wv", [128, 8 * D], BF16, isOutput=False)
    out_ext = nc.declare_dram_parameter("out", [BL, D], F32, isOutput=True)
    dbg = None
    if debug:
        dbg = {
            "q4": nc.declare_dram_parameter("dbg_q4", [BL, D], F32, isOutput=True),
            "r4t": nc.declare_dram_parameter("dbg_r4t", [BL * H, D], F32, isOutput=True),
            "sct": nc.declare_dram_parameter("dbg_sct", [H, NG], F32, isOutput=True),
            "et": nc.declare_dram_parameter("dbg_et", [H, NG], F32, isOutput=True),
            "shat": nc.declare_dram_parameter("dbg_shat", [H, D], F32, isOutput=True),
        }

    with ExitStack() as ctx:
        tc = ctx.enter_context(tile.TileContext(nc))
        _body(ctx, tc, nc, q_ext, kx_ext, v_ext, wq_ext, wkt_ext, wv_ext, out_ext,
              dbg=dbg)
    return nc


def _body(ctx, tc, nc, q_ext, kx_ext, v_ext, wq_ext, wkt_ext, wv_ext, out_ext,
          dbg=None):
    const_pool = ctx.enter_context(tc.tile_pool(name="const", bufs=1))
    r_pool = ctx.enter_context(tc.tile_pool(name="rpool", bufs=1))
    st_pool = ctx.enter_context(tc.tile_pool(name="st", bufs=1))
    wstream = ctx.enter_context(tc.tile_pool(name="wstream", bufs=2))
    kx_pool = ctx.enter_context(tc.tile_pool(name="kxld", bufs=3))
    val_pool = ctx.enter_context(tc.tile_pool(name="valld", bufs=6))
    et_pool = ctx.enter_context(tc.tile_pool(name="etp", bufs=2))
    e_pool = ctx.enter_context(tc.tile_pool(name="ep", bufs=2))
    small_pool = ctx.enter_context(tc.tile_pool(name="small", bufs=1))
    ps_a = ctx.enter_context(tc.tile_pool(name="ps_a", bufs=2, space="PSUM"))
    ps_sc = ctx.enter_context(tc.tile_pool(name="ps_sc", bufs=3, space="PSUM"))
    ps_acc = ctx.enter_context(tc.tile_pool(name="ps_acc", bufs=1, space="PSUM"))

    ident_f = const_pool.tile([128, 128], F32, tag="idf")
    masks.make_identity(nc, ident_f[:])
    ident_b = const_pool.tile([128, 128], BF16, tag="idb")
    masks.make_identity(nc, ident_b[:])

    # ---------------- prologue: q-projection ----------------
    # Single-shot dense weight loads, one per queue so they land in parallel
    # ahead of the key/value streams (per-queue FIFO puts them first).
    wq_sb = wstream.tile([128, 8 * D], BF16, tag="wq", bufs=1)
    nc.sync.dma_start(wq_sb[:], wq_ext[:])
    wkt_sb = wstream.tile([128, 8 * D], BF16, tag="wkt", bufs=1)
    nc.gpsimd.dma_start(wkt_sb[:], wkt_ext[:])

    # query [BL, D] -> qT chunks [128, BL] (contraction dim on partitions)
    q_sb = small_pool.tile([BL, D], F32, tag="q")
    nc.sync.dma_start(q_sb[:], q_ext[:])
    qT = small_pool.tile([128, 8 * BL], BF16, tag="qT")
    for jc in range(8):
        pt = ps_a.tile([128, 128], F32, tag="a")
        nc.tensor.transpose(pt[:, :BL], q_sb[:, jc * 128:(jc + 1) * 128],
                            ident_f[:BL, :BL])
        nc.any.tensor_copy(qT[:, jc * BL:(jc + 1) * BL], pt[:, :BL])

    # q4[b, hk] = sum_j query[b, j] * wq[j, hk]   (all 4 batches at once)
    q4_ps = ps_acc.tile([BL, D], F32, tag="acc")
    for jc in range(8):
        for half in range(2):
            nc.tensor.matmul(q4_ps[:, half * 512:(half + 1) * 512],
                             qT[:, jc * BL:(jc + 1) * BL],
                             wq_sb[:, jc * D + half * 512:jc * D + (half + 1) * 512],
                             start=(jc == 0), stop=(jc == 7))
    q4_sb = small_pool.tile([BL, D], F32, tag="q4")
    nc.any.tensor_copy(q4_sb[:], q4_ps[:])
    if dbg:
        nc.sync.dma_start(dbg["q4"][:], q4_sb[:])

    # q4T chunks: [128 hk, BL]
    q4T = small_pool.tile([128, 8 * BL], BF16, tag="q4T")
    for hc in range(8):
        pt = ps_a.tile([128, 128], F32, tag="a")
        nc.tensor.transpose(pt[:, :BL], q4_sb[:, hc * 128:(hc + 1) * 128],
                            ident_f[:BL, :BL])
        nc.any.tensor_copy(q4T[:, hc * BL:(hc + 1) * BL], pt[:, :BL])

    # Block-diagonal q: Qbd[hk, b*H + h] = q4[b, hk] iff h == hk // 64
    qbd = []
    for hc in range(8):
        qb = small_pool.tile([128, BL * H], BF16, tag=f"qbd{hc}", name=f"qbd{hc}")
        nc.vector.memset(qb[:], 0.0)
        nc.vector.tensor_copy(qb[0:64, 2 * hc:BL * H:H],
                              q4T[0:64, hc * BL:(hc + 1) * BL])
        nc.vector.tensor_copy(qb[64:128, 2 * hc + 1:BL * H:H],
                              q4T[64:128, hc * BL:(hc + 1) * BL])
        qbd.append(qb)

    # R4T[b*H + h, d] = sum_hk Qbd[hk, b*H+h] * wkT[hk, d]
    r4T_ps = ps_acc.tile([BL * H, D], F32, tag="acc")
    for hc in range(8):
        for half in range(2):
            nc.tensor.matmul(r4T_ps[:, half * 512:(half + 1) * 512],
                             qbd[hc][:],
                             wkt_sb[:, hc * D + half * 512:hc * D + (half + 1) * 512],
                             start=(hc == 0), stop=(hc == 7))
    r4T_sb = small_pool.tile([BL * H, D], F32, tag="r4T")
    nc.any.tensor_copy(r4T_sb[:], r4T_ps[:])
    if dbg:
        nc.sync.dma_start(dbg["r4t"][:], r4T_sb[:])

    # Split-fp8 R in DoubleRow layout: per d-chunk, [128 d, 2, BL*H] with
    # ksub 0 = Rhi = fp8(R) and ksub 1 = Rlo = fp8(R - Rhi).  One DoubleRow
    # matmul per chunk then computes (Rhi + Rlo)^T @ kx by feeding the same
    # kx chunk to both ksubs (stride-0 broadcast).
    rcat = [r_pool.tile([128, 2, BL * H], FP8, tag=f"rc{dc}", name=f"rc{dc}")
            for dc in range(8)]
    for dc in range(8):
        pt = ps_a.tile([128, 128], F32, tag="a")
        nc.tensor.transpose(pt[:, :BL * H], r4T_sb[:, dc * 128:(dc + 1) * 128],
                            ident_f[:BL * H, :BL * H])
        nc.vector.tensor_copy(rcat[dc][:, 0, :], pt[:, :BL * H])
        # residual: R - fp8(R), computed in f32 then cast to fp8
        rhi_f = small_pool.tile([128, BL * H], F32, tag="rhif")
        nc.vector.tensor_copy(rhi_f[:], rcat[dc][:, 0, :])
        rlo_f = small_pool.tile([128, BL * H], F32, tag="rlof")
        nc.vector.tensor_sub(rlo_f[:], pt[:, :BL * H], rhi_f[:])
        nc.vector.tensor_copy(rcat[dc][:, 1, :], rlo_f[:])

    # ---------------- main loop ----------------
    sT = [st_pool.tile([128, BL * H], BF16, tag=f"st{dc}", name=f"st{dc}")
          for dc in range(8)]
    n_groups = N // NG                   # 8 groups of 512 rows per batch
    groups_per_kx = NTK // NG            # 4
    wv_sb = None
    for b in range(BL):
        vb = v_ext[b].rearrange("(t four p) d -> t p four d", four=4, p=128)
        # Two consecutive batches share the accumulator banks at partition
        # offsets 0 and 32, so batch b+1 can start accumulating while batch
        # b's epilogue still reads its slice.
        if b % 2 == 0:
            s_pair = ps_acc.tile([32 + H, D], F32, tag="acc", name=f"sacc{b}")
        po = 32 * (b % 2)
        s_ps = s_pair[po:po + H]
        esum = small_pool.tile([H, 1], F32, tag="esum", bufs=2, name=f"esum{b}")
        if b == 2:
            # wv rides the sync queue mid-stream (reusing wq's SBUF, which is
            # dead after the prologue), clear of the value-queue tail
            wv_sb = wstream.tile([128, 8 * D], BF16, tag="wq", bufs=1,
                                 name="wv")
            nc.sync.dma_start(wv_sb[:], wv_ext[:])
        for g in range(n_groups):
            if g % groups_per_kx == 0:
                kx_t = kx_pool.tile([128, 8, NTK], FP8, tag="kx")
                nc.sync.dma_start(
                    kx_t[:],
                    kx_ext[b, g // groups_per_kx].rearrange(
                        "(dc p) r -> p dc r", p=128))
            go = (g % groups_per_kx) * NG    # row offset inside kx tile
            v_t = val_pool.tile([128, 4 * D], BF16, tag="v")
            nc.gpsimd.dma_start(v_t[:].rearrange("p (four d) -> p four d", four=4),
                                vb[g])
            # scores^T [16, 512] in one PSUM bank, one DoubleRow matmul per
            # d-chunk (kx chunk broadcast to both R ksubs)
            scT = ps_sc.tile([H, NG], F32, tag="sc")
            for dc in range(8):
                nc.tensor.matmul(
                    scT[:],
                    rcat[dc][:, :, b * H:(b + 1) * H],
                    kx_t[:, dc, go:go + NG].unsqueeze(1).broadcast_to(
                        (128, 2, NG)),
                    start=(dc == 0), stop=(dc == 7), perf_mode=DR,
                    skip_group_check=True)
            # exp(scores / 8); no max-subtraction needed (|scores| < ~3)
            eT = et_pool.tile([H, NG], BF16, tag="et")
            nc.scalar.activation(eT[:], scT[:], EXP, scale=0.125)
            # running sum of exp on DVE (frees the PE sum-matmuls)
            gsum = small_pool.tile([H, 1], F32, tag="gsum", bufs=2)
            nc.vector.reduce_sum(gsum[:], eT[:], axis=mybir.AxisListType.X)
            if g == 0:
                nc.vector.tensor_copy(esum[:], gsum[:])
            else:
                nc.vector.tensor_add(esum[:], esum[:], gsum[:])
            if dbg and b == 0 and g == 0:
                sct_f = small_pool.tile([H, NG], F32, tag="sctf")
                nc.vector.tensor_copy(sct_f[:], scT[:])
                nc.sync.dma_start(dbg["sct"][:], sct_f[:])
                et_f = small_pool.tile([H, NG], F32, tag="etf")
                nc.vector.tensor_copy(et_f[:], eT[:])
                nc.sync.dma_start(dbg["et"][:], et_f[:])
            # transpose e^T -> e [128, 4*16] (sub, h)
            e_ps = ps_a.tile([128, 4 * H], BF16, tag="a")
            for s in range(4):
                nc.tensor.transpose(e_ps[:, s * H:(s + 1) * H],
                                    eT[:, s * 128:(s + 1) * 128],
                                    ident_b[:H, :H])
            e_sb = e_pool.tile([128, 4 * H], BF16, tag="e")
            nc.vector.tensor_copy(e_sb[:], e_ps[:])
            # attn @ value for the 4 row-subtiles
            for s in range(4):
                first = g == 0 and s == 0
                last = g == n_groups - 1 and s == 3
                e_s = e_sb[:, s * H:(s + 1) * H]
                nc.tensor.matmul(s_ps[:, 0:512], e_s,
                                 v_t[:, s * D:s * D + 512],
                                 start=first, stop=last, skip_group_check=True)
                nc.tensor.matmul(s_ps[:, 512:1024], e_s,
                                 v_t[:, s * D + 512:(s + 1) * D],
                                 start=first, stop=last, skip_group_check=True)
        # batch epilogue: normalize and transpose s
        recip = small_pool.tile([H, 1], F32, tag="recip")
        nc.vector.reciprocal(recip[:], esum[:])
        shat = small_pool.tile([H, D], F32, tag="shat")
        nc.vector.tensor_scalar_mul(shat[:], s_ps[:], recip[:])
        if dbg and b == 0:
            nc.sync.dma_start(dbg["shat"][:], shat[:])
        for dc in range(8):
            pt = ps_a.tile([128, 128], F32, tag="st", name="pt_st", bufs=1)
            nc.tensor.transpose(pt[:, :H], shat[:, dc * 128:(dc + 1) * 128],
                                ident_f[:H, :H])
            nc.any.tensor_copy(sT[dc][:, b * H:(b + 1) * H], pt[:, :H])

    # ---------------- output projection ----------------
    # out[b, h*64+j] = sum_d sT[d, b*H+h] * wv[d, h*64+j]
    # One PSUM tile (= one bank) per head: a start=True matmul clears the
    # has_written bits of its WHOLE bank, so interleaved accumulation groups
    # must never share a bank.
    o_sb = small_pool.tile([BL, D], F32, tag="o")
    for h in range(H):
        oh_ps = ps_sc.tile([BL, 64], F32, tag="sc", name=f"oh{h}")
        for dc in range(8):
            nc.tensor.matmul(oh_ps[:],
                             sT[dc][:, h:BL * H:H],
                             wv_sb[:, dc * D + h * 64:dc * D + (h + 1) * 64],
                             start=(dc == 0), stop=(dc == 7))
        nc.any.tensor_copy(o_sb[:, h * 64:(h + 1) * 64], oh_ps[:])
    nc.sync.dma_start(out_ext[:], o_sb[:])


_graph_cache = {}


def _get_graph():
    if "nc" not in _graph_cache:
        nc = build_graph()
        # Bacc.finalize runs the sync-wait-splitting passes the TRN2 ISA
        # requires (<=1 wait per instruction); the pjrt path serializes the
        # module as-is, so finalize must happen before run.
        if not nc.is_finalized():
            nc.finalize()
        _graph_cache["nc"] = nc
    return _graph_cache["nc"]


def make_in_maps(query, key, value, wq, wk, wv):
    import ml_dtypes
    f = np.float32
    bf = ml_dtypes.bfloat16
    f8 = ml_dtypes.float8_e4m3
    # dtype conversion and layout (transpose/tile) happen host-side as part
    # of sharding; all arithmetic runs on device.
    key8 = np.asarray(key, dtype=np.float32).astype(f8)
    # kx[b, t, d, r] = key[b, t*NTK + r, d]
    kx = np.ascontiguousarray(
        key8.reshape(B, N // NTK, NTK, D).transpose(0, 1, 3, 2))
    value = np.ascontiguousarray(value).astype(bf)

    def pmajor(w):
        # device layout [128, 8*D]: w_dev[p, jc*D + k] = w[jc*128 + p, k]
        return np.ascontiguousarray(
            np.asarray(w, dtype=np.float32).reshape(8, 128, D)
            .transpose(1, 0, 2).reshape(128, 8 * D)).astype(bf)

    wq_b = pmajor(wq)
    wkT = pmajor(np.asarray(wk).T)
    wv_b = pmajor(wv)
    maps = []
    for c in range(N_CORES):
        sl = slice(c * BL, (c + 1) * BL)
        maps.append({
            "query": np.ascontiguousarray(query[sl], dtype=f),
            "kx": np.ascontiguousarray(kx[sl]),
            "value": np.ascontiguousarray(value[sl]),
            "wq": wq_b, "wkT": wkT, "wv": wv_b,
        })
    return maps


def kernel(query, key, value, wq, wk, wv):
    nc = _get_graph()
    in_maps = make_in_maps(query, key, value, wq, wk, wv)
    res = run_bass_kernel_spmd(nc, in_maps, core_ids=list(range(N_CORES)))
    out = np.concatenate([r["out"] for r in res.results], axis=0)
    return out.astype(np.float32)
